# revision 9
# baseline (speedup 1.0000x reference)
"""Bass/Trainium2 kernel for nn_DecoderRNN: feedback LSTM decoder.

Math per step (PyTorch LSTMCell, gates (i,f,g,o)):
    gates = x @ W_ih.T + b_ih + h @ W_hh.T + b_hh     x = prev softmax output
    c' = sig(f)*c + sig(i)*tanh(g);  h' = sig(o)*tanh(c')
    y  = softmax(h' @ W_out.T + b_out);  x_next = y
Output is time-reversed: out[T-1-t] = y_t.

Sharding: data-parallel over batch across 8 cores (B=512 -> 64/core),
weights replicated, recurrence local per core.

Device-side design (per core, B=64):
- "H-folded" layout: every per-gate [B, 1024] tensor is stored as
  [128, 512] with partition p = j*64 + b (j = h-half).  This fills all
  128 partitions/PE columns even though the per-core batch is only 64,
  and keeps every elementwise op lane-local.
- gates are computed per-gate (chunks of 512 h-features x 2 halves):
  stationary = xT/hT k-tiles [128, 64]; the two h-halves run as
  concurrent column-group matmuls (tile positions (0,0) / (0,64))
  accumulating into one PSUM bank.  (HW-measured: a pair costs ~257ns
  vs 241ns for one serial MM -> ~1.85x concurrency.)
- logits are ALSO folded: pl [128, 256] with p = jO*64 + b (jO =
  O-half), so each hT k-tile drives a col-group PAIR of N=256 streams.
  Softmax then needs a cross-half sum: ssum[0:64] + ssum[64:128].
- h'/y stay folded; transposes back to [feature, batch] are full
  [128,128] PE transposes (4 for h, 2 for y vs 8+4 thin ones before).
  One [128,128] transpose of folded data yields TWO k-tiles (h-halves
  interleave), so hT/xT column order is permuted: korder [0,4,1,5,...]
  (h) / [0,2,1,3] (x); weight layouts are permuted to match host-side.
- gate order permuted to (i, f, o, g); sigmoid computed as
  0.5 + 0.5*tanh(x/2) so only the exp_and_others ACT table set is used
  (tanh + exp; no table swaps).
- b_ih+b_hh folded into W_ih.T rows (softmax x sums to exactly 1, so
  adding b to every row of W_ih.T adds b*sum(x) = b).  Step 0 has x=0,
  so its bias comes from K=1 ones-matmuls against a bias row instead.
- b_out added via a K=1 ones-matmul pair into the logits PSUM.
"""

import numpy as np
import ml_dtypes

B = 64          # batch per core
H = 1024
HF = 512        # folded h-half size
O = 512
OF = 256        # folded O-half size
G = 4 * H       # 4096
T = 256
KH = H // 128   # 8 h k-tiles
KX = O // 128   # 4 x k-tiles
NCORES = 8

KORDER = [0, 4, 1, 5, 2, 6, 3, 7]   # hT column-block -> h k-tile
KXORDER = [0, 2, 1, 3]              # xT column-block -> x k-tile

_BF16 = ml_dtypes.bfloat16

_cache = {}

# Number of steps actually emitted (out buffer stays [T, B, O]); test
# harnesses may lower this to build a transfer-identical baseline module.
T_LIVE = None
# When set (int R), wraps the whole step loop in a hardware For_i loop so
# the body executes R times — used to measure per-step time above host
# noise.  Output values are garbage after the first iteration.
TIMING_REPS = None


def _build():
    import concourse.bass as bass
    import concourse.tile as tile
    from concourse import bacc, mybir

    f32 = mybir.dt.float32
    bf16 = mybir.dt.bfloat16
    Tanh = mybir.ActivationFunctionType.Tanh
    Exp = mybir.ActivationFunctionType.Exp

    nc = bacc.Bacc("TRN2", target_bir_lowering=False, debug=False,
                   num_devices=NCORES)

    # ---- DRAM I/O ----
    # wih: [128, KX*4*2*512]  (kx-block, gate, h-half, h-col), bias folded
    # whh: [128, KH*4*2*512]  (k-block, gate, h-half, h-col)
    # wout: [128, KH*2*256]   (k-block, O-half, O-col)
    wih_d = nc.dram_tensor("wih", [128, KX * G], bf16, kind="ExternalInput")
    whh_d = nc.dram_tensor("whh", [128, KH * G], bf16, kind="ExternalInput")
    wout_d = nc.dram_tensor("wout", [128, KH * O], bf16, kind="ExternalInput")
    biasrow_d = nc.dram_tensor("biasrow", [1, G], bf16, kind="ExternalInput")
    boutrow_d = nc.dram_tensor("boutrow", [1, O], bf16, kind="ExternalInput")
    onesrow_d = nc.dram_tensor("onesrow", [1, B], bf16, kind="ExternalInput")
    ident_d = nc.dram_tensor("ident", [128, 128], bf16, kind="ExternalInput")
    h0t_d = nc.dram_tensor("h0t", [128, KH * B], bf16, kind="ExternalInput")
    c0_d = nc.dram_tensor("c0", [128, HF], f32, kind="ExternalInput")
    out_d = nc.dram_tensor("out", [T, B, O], f32, kind="ExternalOutput")

    with tile.TileContext(nc) as tc:
        with (
            tc.tile_pool(name="consts", bufs=1) as consts,
            tc.tile_pool(name="state_c", bufs=2) as state_c,
            tc.tile_pool(name="state_ht", bufs=2) as state_ht,
            tc.tile_pool(name="state_xt", bufs=2) as state_xt,
            tc.tile_pool(name="work", bufs=2) as work,
            tc.tile_pool(name="ys", bufs=3) as ys,
            tc.tile_pool(name="psum_g", bufs=4, space="PSUM") as psum_g,
            tc.tile_pool(name="psum_l", bufs=1, space="PSUM") as psum_l,
            tc.tile_pool(name="psum_t", bufs=3, space="PSUM") as psum_t,
        ):
            # ---- load constants ----
            wih = consts.tile([128, KX * G], bf16)
            nc.sync.dma_start(out=wih, in_=wih_d[:, :])
            whh = consts.tile([128, KH * G], bf16)
            nc.sync.dma_start(out=whh, in_=whh_d[:, :])
            wout = consts.tile([128, KH * O], bf16)
            nc.sync.dma_start(out=wout, in_=wout_d[:, :])
            biasrow = consts.tile([1, G], bf16)
            nc.sync.dma_start(out=biasrow, in_=biasrow_d[:, :])
            boutrow = consts.tile([1, O], bf16)
            nc.sync.dma_start(out=boutrow, in_=boutrow_d[:, :])
            onesrow = consts.tile([1, B], bf16)
            nc.sync.dma_start(out=onesrow, in_=onesrow_d[:, :])
            ident = consts.tile([128, 128], bf16)
            nc.sync.dma_start(out=ident, in_=ident_d[:, :])

            c_prev = state_c.tile([128, HF], f32, tag="c")
            nc.sync.dma_start(out=c_prev, in_=c0_d[:, :])
            hT_prev = state_ht.tile([128, KH * B], bf16, tag="ht")
            nc.sync.dma_start(out=hT_prev, in_=h0t_d[:, :])
            xT_prev = None

            def wslice(w, i, g, j):
                # weight block for k-position i, gate g, h-half j: [128, 512]
                base = ((i * 4 + g) * 2 + j) * HF
                return w[:, base:base + HF]

            def _emit_logits(nc, ptrs, hT_new, pl, wout, ch):
                # copy transpose-psum chunk ch into hT, then its two
                # k-positions' logits col-pair matmuls
                nc.vector.tensor_copy(
                    out=hT_new[:, 2 * ch * B:(2 * ch + 2) * B],
                    in_=ptrs[ch])
                for i in (2 * ch, 2 * ch + 1):
                    lastk = i == KH - 1
                    nc.tensor.matmul(
                        pl[0:B, :], hT_new[:, i * B:(i + 1) * B],
                        wout[:, i * O:i * O + OF],
                        start=False, stop=lastk,
                        skip_group_check=True)
                    nc.tensor.matmul(
                        pl[B:128, :], hT_new[:, i * B:(i + 1) * B],
                        wout[:, i * O + OF:(i + 1) * O],
                        start=False, stop=lastk,
                        skip_group_check=True)

            t_live = T if T_LIVE is None else T_LIVE
            from contextlib import nullcontext
            loop_ctx = (tc.For_i(0, int(TIMING_REPS), 1)
                        if TIMING_REPS else nullcontext())
            with loop_ctx:
                pend_ybf = None
                for t in range(t_live):
                    # ---------------- gates: h-part (all 4 chunks) ------
                    # emission order (o, i, g, f): f last => only f's
                    # act->sig->c->h chain is on the step-boundary critical
                    # path; o/i/g activations and u2 run during f's matmuls.
                    tg = work.tile([128, 4, HF], bf16, tag="tg")
                    sg = work.tile([128, 3, HF], bf16, tag="sg")
                    pgs = {}
                    for g in (2, 0, 3, 1):
                        pg = psum_g.tile([128, HF], f32, tag="pg")
                        pgs[g] = pg
                        for i in range(KH):
                            nc.tensor.matmul(pg[0:B, :],
                                             hT_prev[:, i * B:(i + 1) * B],
                                             wslice(whh, i, g, 0),
                                             start=(i == 0), stop=False,
                                             skip_group_check=True)
                            nc.tensor.matmul(pg[B:128, :],
                                             hT_prev[:, i * B:(i + 1) * B],
                                             wslice(whh, i, g, 1),
                                             start=(i == 0), stop=False,
                                             skip_group_check=True)

                    # ---- previous step's y_bf -> xT transposes ----
                    if pend_ybf is not None:
                        ybf = pend_ybf
                        xT_new = state_xt.tile([128, KX * B], bf16,
                                               tag="xt")
                        for cx in range(2):
                            ptry = psum_t.tile([128, 128], bf16, tag="ptr")
                            nc.tensor.transpose(
                                ptry,
                                ybf[:, cx * 128:(cx + 1) * 128], ident)
                            nc.vector.tensor_copy(
                                out=xT_new[:, cx * 128:(cx + 1) * 128],
                                in_=ptry)
                        xT_prev = xT_new
                        pend_ybf = None

                    # ---------------- gates: x-part + activations -------
                    for g in (2, 0, 3, 1):
                        pg = pgs[g]
                        chain = g == 1   # f: last gate, on critical path
                        if t == 0:  # bias via K=1 ones-matmul
                            b0 = (g * 2) * HF
                            nc.tensor.matmul(pg[0:B, :], onesrow,
                                             biasrow[:, b0:b0 + HF],
                                             start=False, stop=True,
                                             skip_group_check=True)
                            nc.tensor.matmul(pg[B:128, :], onesrow,
                                             biasrow[:, b0 + HF:b0 + 2 * HF],
                                             start=False, stop=True,
                                             skip_group_check=True)
                        else:
                            for i in range(KX):
                                last = i == KX - 1
                                if chain and last:
                                    # split the final accumulating pair by
                                    # column halves so act(f) starts early
                                    for jj in range(2):
                                        hs = slice(jj * 256, (jj + 1) * 256)
                                        nc.tensor.matmul(
                                            pg[0:B, hs],
                                            xT_prev[:, i * B:(i + 1) * B],
                                            wslice(wih, i, g, 0)[:, hs],
                                            start=False, stop=True,
                                            skip_group_check=True)
                                        nc.tensor.matmul(
                                            pg[B:128, hs],
                                            xT_prev[:, i * B:(i + 1) * B],
                                            wslice(wih, i, g, 1)[:, hs],
                                            start=False, stop=True,
                                            skip_group_check=True)
                                    continue
                                nc.tensor.matmul(
                                    pg[0:B, :],
                                    xT_prev[:, i * B:(i + 1) * B],
                                    wslice(wih, i, g, 0),
                                    start=False, stop=last,
                                    skip_group_check=True)
                                nc.tensor.matmul(
                                    pg[B:128, :],
                                    xT_prev[:, i * B:(i + 1) * B],
                                    wslice(wih, i, g, 1),
                                    start=False, stop=last,
                                    skip_group_check=True)
                        # tanh for this gate (x/2 for i,f,o; g==3 is plain)
                        # chain gate f at 128-col chunks so downstream
                        # c/h chunks pipeline; others full width.
                        if chain:
                            for ch in range(4):
                                cs = slice(ch * 128, (ch + 1) * 128)
                                nc.scalar.activation(
                                    out=tg[:, g, cs], in_=pg[:, cs],
                                    func=Tanh, scale=0.5)
                        else:
                            nc.scalar.activation(
                                out=tg[:, g, :], in_=pg, func=Tanh,
                                scale=0.5 if g != 3 else 1.0)
                            if g != 3:  # sigmoid:  s = 0.5*tanh + 0.5
                                nc.vector.tensor_scalar(
                                    out=sg[:, g, :], in0=tg[:, g, :],
                                    scalar1=0.5, scalar2=0.5,
                                    op0=mybir.AluOpType.mult,
                                    op1=mybir.AluOpType.add)

                    # ------- c / h update at 128-col chunk granularity ---
                    # sig(f)*c + sig(i)*tanh(g)
                    #   = tanh(f/2)*(c/2) + [(c/2) + sig(i)*tanh(g)]
                    # ch_ and w are computed off the critical chain, so the
                    # per-chunk chain is tanh(f/2) -> mul -> add -> tanh ->
                    # mul -> transpose, with no sigmoid affine in it.
                    ch_ = work.tile([128, HF], f32, tag="ch_")
                    nc.vector.tensor_scalar_mul(out=ch_, in0=c_prev,
                                                scalar1=0.5)
                    u2 = work.tile([128, HF], bf16, tag="u2")
                    nc.gpsimd.tensor_mul(out=u2, in0=sg[:, 0, :],
                                         in1=tg[:, 3, :])
                    w = work.tile([128, HF], f32, tag="w")
                    nc.gpsimd.tensor_add(out=w, in0=ch_, in1=u2)
                    u1 = work.tile([128, HF], f32, tag="u1")
                    c_new = state_c.tile([128, HF], f32, tag="c")
                    th = work.tile([128, HF], bf16, tag="th")
                    hn = work.tile([128, HF], bf16, tag="hn")

                    # logits psum: bias pair opens the accumulation
                    pl = psum_l.tile([128, OF], f32, tag="pl")
                    nc.tensor.matmul(pl[0:B, :], onesrow, boutrow[:, 0:OF],
                                     start=True, stop=False,
                                     skip_group_check=True)
                    nc.tensor.matmul(pl[B:128, :], onesrow,
                                     boutrow[:, OF:O],
                                     start=True, stop=False,
                                     skip_group_check=True)

                    hT_new = state_ht.tile([128, KH * B], bf16, tag="ht")
                    ptrs = {}
                    for ch in range(4):
                        cs = slice(ch * 128, (ch + 1) * 128)
                        nc.vector.tensor_mul(out=u1[:, cs],
                                             in0=tg[:, 1, cs],
                                             in1=ch_[:, cs])
                        nc.vector.tensor_add(out=c_new[:, cs],
                                             in0=u1[:, cs], in1=w[:, cs])
                        nc.scalar.activation(out=th[:, cs],
                                             in_=c_new[:, cs], func=Tanh)
                        nc.vector.tensor_mul(out=hn[:, cs],
                                             in0=sg[:, 2, cs],
                                             in1=th[:, cs])
                        # [128,128] transpose yields hT column-blocks
                        # (2ch, 2ch+1) = k-tiles (ch, ch+4).  PE order is
                        # T0 T1 L0 T2 L1 T3 L2 L3 so the DVE copy for
                        # chunk n hides under transpose n+1.
                        ptrh = psum_t.tile([128, 128], bf16, tag="ptr")
                        nc.tensor.transpose(
                            ptrh, hn[:, cs], ident)
                        ptrs[ch] = ptrh
                        if ch >= 1:
                            _emit_logits(nc, ptrs, hT_new, pl, wout, ch - 1)
                    _emit_logits(nc, ptrs, hT_new, pl, wout, 3)

                    # ---------------- softmax (folded) ----------------
                    eu = work.tile([128, OF], f32, tag="eu")
                    ssum = work.tile([128, 1], f32, tag="ssum")
                    nc.scalar.activation(out=eu, in_=pl, func=Exp,
                                         accum_out=ssum)
                    stmp = work.tile([B, 1], f32, tag="stmp")
                    nc.vector.tensor_copy(out=stmp, in_=ssum[B:128])
                    sden = work.tile([B, 1], f32, tag="sden")
                    nc.vector.tensor_add(out=sden, in0=ssum[0:B],
                                         in1=stmp)
                    sinv = work.tile([128, 1], f32, tag="sinv")
                    nc.vector.reciprocal(out=sinv[0:B], in_=sden)
                    nc.vector.reciprocal(out=sinv[B:128], in_=sden)
                    y = ys.tile([128, OF], f32, tag="y")
                    nc.scalar.mul(out=y[:, 0:128], in_=eu[:, 0:128],
                                  mul=sinv)
                    nc.scalar.mul(out=y[:, 128:OF], in_=eu[:, 128:OF],
                                  mul=sinv)
                    trow = (T - 1 - t) % T
                    nc.sync.dma_start(out=out_d[trow, :, 0:OF],
                                      in_=y[0:B, :])
                    nc.sync.dma_start(out=out_d[trow, :, OF:O],
                                      in_=y[B:128, :])
                    if t < t_live - 1 or TIMING_REPS:
                        ybf = ys.tile([128, OF], bf16, tag="ybf")
                        nc.scalar.mul(out=ybf, in_=eu, mul=sinv)
                        pend_ybf = ybf

                    c_prev = c_new
                    hT_prev = hT_new

    nc.compile()
    return nc


def _host_prep(h0, c0, W_ih, W_hh, b_ih, b_hh, W_out, b_out):
    """Build per-core input maps (host-side layout transforms)."""
    f32 = np.float32
    h0 = np.asarray(h0, f32).reshape(NCORES * B, H)
    c0 = np.asarray(c0, f32).reshape(NCORES * B, H)
    W_ih = np.asarray(W_ih, f32)
    W_hh = np.asarray(W_hh, f32)
    W_out = np.asarray(W_out, f32)
    b_tot = np.asarray(b_ih, f32) + np.asarray(b_hh, f32)
    b_out = np.asarray(b_out, f32)

    # permute gate order (i, f, g, o) -> (i, f, o, g)
    perm = np.r_[0:H, H:2 * H, 3 * H:4 * H, 2 * H:3 * H]
    Wih_p = W_ih[perm]          # [G, O]
    Whh_p = W_hh[perm]          # [G, H]
    b_p = b_tot[perm]           # [G]

    # weight layout: [p, kpos, gate, h-half, h-col] flattened to [128, K*G],
    # with k-tiles permuted so transposed activations land in order.
    WihT_aug = Wih_p.T + b_p[None, :]           # [O, G]
    wih_host = np.ascontiguousarray(
        WihT_aug.reshape(KX, 128, 4, 2, HF)[KXORDER].transpose(1, 0, 2, 3, 4)
    ).reshape(128, KX * G).astype(_BF16)
    whh_host = np.ascontiguousarray(
        Whh_p.T.reshape(KH, 128, 4, 2, HF)[KORDER].transpose(1, 0, 2, 3, 4)
    ).reshape(128, KH * G).astype(_BF16)
    # wout blocks: [kpos, O-half, O-col]
    wout_host = np.ascontiguousarray(
        W_out.T.reshape(KH, 128, O)[KORDER].transpose(1, 0, 2)
    ).reshape(128, KH * O).astype(_BF16)
    biasrow = b_p[None, :].astype(_BF16)        # [1, (gate, half, col)]
    boutrow = b_out[None, :].astype(_BF16)
    onesrow = np.ones((1, B), _BF16)
    ident = np.eye(128).astype(_BF16)

    in_maps = []
    for i in range(NCORES):
        sl = slice(i * B, (i + 1) * B)
        h0s = h0[sl]                                # [B, H]
        h0t = np.ascontiguousarray(
            h0s.reshape(B, KH, 128).transpose(1, 2, 0)[KORDER]
        ).reshape(KH, 128, B).transpose(1, 0, 2).reshape(128, KH * B)
        c0f = np.ascontiguousarray(
            c0[sl].reshape(B, 2, HF).transpose(1, 0, 2)).reshape(128, HF)
        in_maps.append({
            "wih": wih_host, "whh": whh_host, "wout": wout_host,
            "biasrow": biasrow, "boutrow": boutrow, "onesrow": onesrow,
            "ident": ident,
            "h0t": np.ascontiguousarray(h0t).astype(_BF16),
            "c0": c0f,
        })
    return in_maps


def kernel(h0, c0, W_ih, W_hh, b_ih, b_hh, W_out, b_out, out_len):
    from concourse.bass_utils import run_bass_kernel_spmd

    assert int(out_len) == T
    if "nc" not in _cache:
        _cache["nc"] = _build()
    nc = _cache["nc"]
    in_maps = _host_prep(h0, c0, W_ih, W_hh, b_ih, b_hh, W_out, b_out)
    res = run_bass_kernel_spmd(nc, in_maps, core_ids=list(range(NCORES)))
    full = np.empty((T, NCORES * B, O), np.float32)
    for i in range(NCORES):
        full[:, i * B:(i + 1) * B, :] = res.results[i]["out"]
    return full


# revision 13
# speedup vs baseline: 1.0769x; 1.0769x over previous
"""Bass/Trainium2 kernel for nn_DecoderRNN: feedback LSTM decoder.

Math per step (PyTorch LSTMCell, gates (i,f,g,o)):
    gates = x @ W_ih.T + b_ih + h @ W_hh.T + b_hh     x = prev softmax output
    c' = sig(f)*c + sig(i)*tanh(g);  h' = sig(o)*tanh(c')
    y  = softmax(h' @ W_out.T + b_out);  x_next = y
Output is time-reversed: out[T-1-t] = y_t.

Sharding: data-parallel over batch across 8 cores (B=512 -> 64/core),
weights replicated, recurrence local per core.

Device-side design (per core, B=64):
- "H-folded" layout: every per-gate [B, 1024] tensor is stored as
  [128, 512] with partition p = j*64 + b (j = h-half).  This fills all
  128 partitions/PE columns even though the per-core batch is only 64,
  and keeps every elementwise op lane-local.
- gates are computed per-gate (chunks of 512 h-features x 2 halves):
  stationary = xT/hT k-tiles [128, 64]; the two h-halves run as
  concurrent column-group matmuls (tile positions (0,0) / (0,64))
  accumulating into one PSUM bank.  (HW-measured: a pair costs ~257ns
  vs 241ns for one serial MM -> ~1.85x concurrency.)
- logits are ALSO folded: pl [128, 256] with p = jO*64 + b (jO =
  O-half), so each hT k-tile drives a col-group PAIR of N=256 streams.
  Softmax then needs a cross-half sum: ssum[0:64] + ssum[64:128].
- h'/y stay folded; transposes back to [feature, batch] are full
  [128,128] PE transposes (4 for h, 2 for y vs 8+4 thin ones before).
  One [128,128] transpose of folded data yields TWO k-tiles (h-halves
  interleave), so hT/xT column order is permuted: korder [0,4,1,5,...]
  (h) / [0,2,1,3] (x); weight layouts are permuted to match host-side.
- gate order permuted to (i, f, o, g); sigmoid computed as
  0.5 + 0.5*tanh(x/2) so only the exp_and_others ACT table set is used
  (tanh + exp; no table swaps).
- b_ih+b_hh folded into W_ih.T rows (softmax x sums to exactly 1, so
  adding b to every row of W_ih.T adds b*sum(x) = b).  Step 0 has x=0,
  so its bias comes from K=1 ones-matmuls against a bias row instead.
- b_out added via a K=1 ones-matmul pair into the logits PSUM.
"""

import numpy as np
import ml_dtypes

B = 64          # batch per core
H = 1024
HF = 512        # folded h-half size
O = 512
OF = 256        # folded O-half size
G = 4 * H       # 4096
T = 256
KH = H // 128   # 8 h k-tiles
KX = O // 128   # 4 x k-tiles
NCORES = 8

KORDER = [0, 4, 1, 5, 2, 6, 3, 7]   # hT column-block -> h k-tile
KXORDER = [0, 2, 1, 3]              # xT column-block -> x k-tile

_BF16 = ml_dtypes.bfloat16

_cache = {}

# Number of steps actually emitted (out buffer stays [T, B, O]); test
# harnesses may lower this to build a transfer-identical baseline module.
T_LIVE = None
# When set (int R), wraps the whole step loop in a hardware For_i loop so
# the body executes R times — used to measure per-step time above host
# noise.  Output values are garbage after the first iteration.
TIMING_REPS = None


def _build():
    import concourse.bass as bass
    import concourse.tile as tile
    from concourse import bacc, mybir

    f32 = mybir.dt.float32
    bf16 = mybir.dt.bfloat16
    Tanh = mybir.ActivationFunctionType.Tanh
    Exp = mybir.ActivationFunctionType.Exp

    nc = bacc.Bacc("TRN2", target_bir_lowering=False, debug=False,
                   num_devices=NCORES)

    # ---- DRAM I/O ----
    # wih: [128, KX*4*2*512]  (kx-block, gate, h-half, h-col), bias folded
    # whh: [128, KH*4*2*512]  (k-block, gate, h-half, h-col)
    # wout: [128, KH*2*256]   (k-block, O-half, O-col)
    wih_d = nc.dram_tensor("wih", [128, KX * G], bf16, kind="ExternalInput")
    whh_d = nc.dram_tensor("whh", [128, KH * G], bf16, kind="ExternalInput")
    wout_d = nc.dram_tensor("wout", [128, KH * O], bf16, kind="ExternalInput")
    biasrow_d = nc.dram_tensor("biasrow", [1, G], bf16, kind="ExternalInput")
    boutrow_d = nc.dram_tensor("boutrow", [1, O], bf16, kind="ExternalInput")
    onesrow_d = nc.dram_tensor("onesrow", [1, B], bf16, kind="ExternalInput")
    ident_d = nc.dram_tensor("ident", [128, 128], bf16, kind="ExternalInput")
    h0t_d = nc.dram_tensor("h0t", [128, KH * B], bf16, kind="ExternalInput")
    c0_d = nc.dram_tensor("c0", [128, HF], f32, kind="ExternalInput")
    out_d = nc.dram_tensor("out", [T, B, O], f32, kind="ExternalOutput")

    with tile.TileContext(nc) as tc:
        with (
            tc.tile_pool(name="consts", bufs=1) as consts,
            tc.tile_pool(name="state_c", bufs=2) as state_c,
            tc.tile_pool(name="state_ht", bufs=2) as state_ht,
            tc.tile_pool(name="state_xt", bufs=2) as state_xt,
            tc.tile_pool(name="work", bufs=2) as work,
            tc.tile_pool(name="ys", bufs=3) as ys,
            tc.tile_pool(name="psum_g", bufs=4, space="PSUM") as psum_g,
            tc.tile_pool(name="psum_l", bufs=1, space="PSUM") as psum_l,
            tc.tile_pool(name="psum_t", bufs=3, space="PSUM") as psum_t,
        ):
            # ---- load constants ----
            wih = consts.tile([128, KX * G], bf16)
            nc.sync.dma_start(out=wih, in_=wih_d[:, :])
            whh = consts.tile([128, KH * G], bf16)
            nc.sync.dma_start(out=whh, in_=whh_d[:, :])
            wout = consts.tile([128, KH * O], bf16)
            nc.sync.dma_start(out=wout, in_=wout_d[:, :])
            biasrow = consts.tile([1, G], bf16)
            nc.sync.dma_start(out=biasrow, in_=biasrow_d[:, :])
            boutrow = consts.tile([1, O], bf16)
            nc.sync.dma_start(out=boutrow, in_=boutrow_d[:, :])
            onesrow = consts.tile([1, B], bf16)
            nc.sync.dma_start(out=onesrow, in_=onesrow_d[:, :])
            ident = consts.tile([128, 128], bf16)
            nc.sync.dma_start(out=ident, in_=ident_d[:, :])

            c_prev = state_c.tile([128, HF], f32, tag="c")
            nc.sync.dma_start(out=c_prev, in_=c0_d[:, :])
            hT_prev = state_ht.tile([128, KH * B], bf16, tag="ht")
            nc.sync.dma_start(out=hT_prev, in_=h0t_d[:, :])
            xT_prev = None

            def wslice(w, i, g, j):
                # weight block for k-position i, gate g, h-half j: [128, 512]
                base = ((i * 4 + g) * 2 + j) * HF
                return w[:, base:base + HF]

            def _emit_logits(nc, ptrs, hT_new, pl, wout, ch):
                # copy transpose-psum chunk ch into hT, then its two
                # k-positions' logits col-pair matmuls
                nc.vector.tensor_copy(
                    out=hT_new[:, 2 * ch * B:(2 * ch + 2) * B],
                    in_=ptrs[ch])
                for i in (2 * ch, 2 * ch + 1):
                    lastk = i == KH - 1
                    nc.tensor.matmul(
                        pl[0:B, :], hT_new[:, i * B:(i + 1) * B],
                        wout[:, i * O:i * O + OF],
                        start=False, stop=lastk,
                        skip_group_check=True)
                    nc.tensor.matmul(
                        pl[B:128, :], hT_new[:, i * B:(i + 1) * B],
                        wout[:, i * O + OF:(i + 1) * O],
                        start=False, stop=lastk,
                        skip_group_check=True)

            t_live = T if T_LIVE is None else T_LIVE
            from contextlib import nullcontext
            loop_ctx = (tc.For_i(0, int(TIMING_REPS), 1)
                        if TIMING_REPS else nullcontext())
            with loop_ctx:
                pend_ybf = None
                for t in range(t_live):
                    # ---------------- gates: h-part (all 4 chunks) ------
                    # emission order (o, i, g, f): f last => only f's
                    # act->sig->c->h chain is on the step-boundary critical
                    # path; o/i/g activations and u2 run during f's matmuls.
                    tg = work.tile([128, 4, HF], bf16, tag="tg")
                    sg = work.tile([128, 3, HF], bf16, tag="sg")
                    ch_ = work.tile([128, HF], f32, tag="ch_")
                    nc.vector.tensor_scalar_mul(out=ch_, in0=c_prev,
                                                scalar1=0.5)
                    pgs = {}
                    for g in (2, 0, 3, 1):
                        pg = psum_g.tile([128, HF], f32, tag="pg")
                        pgs[g] = pg
                        for i in range(KH):
                            nc.tensor.matmul(pg[0:B, :],
                                             hT_prev[:, i * B:(i + 1) * B],
                                             wslice(whh, i, g, 0),
                                             start=(i == 0), stop=False,
                                             skip_group_check=True)
                            nc.tensor.matmul(pg[B:128, :],
                                             hT_prev[:, i * B:(i + 1) * B],
                                             wslice(whh, i, g, 1),
                                             start=(i == 0), stop=False,
                                             skip_group_check=True)

                    # ---- previous step's y_bf -> xT transposes ----
                    if pend_ybf is not None:
                        ybf = pend_ybf
                        xT_new = state_xt.tile([128, KX * B], bf16,
                                               tag="xt")
                        for cx in range(2):
                            ptry = psum_t.tile([128, 128], bf16, tag="ptr")
                            nc.tensor.transpose(
                                ptry,
                                ybf[:, cx * 128:(cx + 1) * 128], ident)
                            nc.vector.tensor_copy(
                                out=xT_new[:, cx * 128:(cx + 1) * 128],
                                in_=ptry)
                        xT_prev = xT_new
                        pend_ybf = None

                    # ---------------- gates: x-part + activations -------
                    for g in (2, 0, 3, 1):
                        pg = pgs[g]
                        chain = g == 1   # f: last gate, on critical path
                        if t == 0:  # bias via K=1 ones-matmul
                            b0 = (g * 2) * HF
                            nc.tensor.matmul(pg[0:B, :], onesrow,
                                             biasrow[:, b0:b0 + HF],
                                             start=False, stop=True,
                                             skip_group_check=True)
                            nc.tensor.matmul(pg[B:128, :], onesrow,
                                             biasrow[:, b0 + HF:b0 + 2 * HF],
                                             start=False, stop=True,
                                             skip_group_check=True)
                        else:
                            for i in range(KX):
                                last = i == KX - 1
                                if chain and last:
                                    # split the final accumulating pair by
                                    # column halves so act(f) starts early
                                    for jj in range(2):
                                        hs = slice(jj * 256, (jj + 1) * 256)
                                        nc.tensor.matmul(
                                            pg[0:B, hs],
                                            xT_prev[:, i * B:(i + 1) * B],
                                            wslice(wih, i, g, 0)[:, hs],
                                            start=False, stop=True,
                                            skip_group_check=True)
                                        nc.tensor.matmul(
                                            pg[B:128, hs],
                                            xT_prev[:, i * B:(i + 1) * B],
                                            wslice(wih, i, g, 1)[:, hs],
                                            start=False, stop=True,
                                            skip_group_check=True)
                                    continue
                                nc.tensor.matmul(
                                    pg[0:B, :],
                                    xT_prev[:, i * B:(i + 1) * B],
                                    wslice(wih, i, g, 0),
                                    start=False, stop=last,
                                    skip_group_check=True)
                                nc.tensor.matmul(
                                    pg[B:128, :],
                                    xT_prev[:, i * B:(i + 1) * B],
                                    wslice(wih, i, g, 1),
                                    start=False, stop=last,
                                    skip_group_check=True)
                        # tanh for this gate (x/2 for i,f,o; g==3 is plain)
                        # chain gate f at column halves so downstream c/h
                        # halves pipeline; others full width.
                        if chain:
                            for q in range(2):
                                cs = slice(q * 256, (q + 1) * 256)
                                nc.scalar.activation(
                                    out=tg[:, g, cs], in_=pg[:, cs],
                                    func=Tanh, scale=0.5)
                        else:
                            nc.scalar.activation(
                                out=tg[:, g, :], in_=pg, func=Tanh,
                                scale=0.5 if g != 3 else 1.0)
                            if g != 3:  # sigmoid:  s = 0.5*tanh + 0.5
                                nc.vector.tensor_scalar(
                                    out=sg[:, g, :], in0=tg[:, g, :],
                                    scalar1=0.5, scalar2=0.5,
                                    op0=mybir.AluOpType.mult,
                                    op1=mybir.AluOpType.add)
                        if g == 3:
                            # u2/w computed off-chain during f's matmuls:
                            # sig(f)*c + sig(i)*tanh(g)
                            #   = tanh(f/2)*(c/2) + [(c/2) + sig(i)*tanh(g)]
                            u2 = work.tile([128, HF], bf16, tag="u2")
                            nc.vector.tensor_mul(out=u2, in0=sg[:, 0, :],
                                                 in1=tg[:, 3, :])
                            w = work.tile([128, HF], f32, tag="w")
                            nc.gpsimd.tensor_add(out=w, in0=ch_, in1=u2)

                    # ------- c / h update, split in column halves -------
                    u1 = work.tile([128, HF], f32, tag="u1")
                    c_new = state_c.tile([128, HF], f32, tag="c")
                    th = work.tile([128, HF], bf16, tag="th")
                    hn = work.tile([128, HF], bf16, tag="hn")

                    # logits psum: bias pair opens the accumulation
                    pl = psum_l.tile([128, OF], f32, tag="pl")
                    nc.tensor.matmul(pl[0:B, :], onesrow, boutrow[:, 0:OF],
                                     start=True, stop=False,
                                     skip_group_check=True)
                    nc.tensor.matmul(pl[B:128, :], onesrow,
                                     boutrow[:, OF:O],
                                     start=True, stop=False,
                                     skip_group_check=True)

                    hT_new = state_ht.tile([128, KH * B], bf16, tag="ht")
                    HQ = HF // 2
                    for q in range(2):
                        cs = slice(q * HQ, (q + 1) * HQ)
                        nc.vector.tensor_mul(out=u1[:, cs],
                                             in0=tg[:, 1, cs],
                                             in1=ch_[:, cs])
                        nc.vector.tensor_add(out=c_new[:, cs],
                                             in0=u1[:, cs], in1=w[:, cs])
                        nc.scalar.activation(out=th[:, cs],
                                             in_=c_new[:, cs], func=Tanh)
                        nc.vector.tensor_mul(out=hn[:, cs],
                                             in0=sg[:, 2, cs],
                                             in1=th[:, cs])
                        # two [128,128] transposes per half; each yields
                        # hT column-blocks (2ch, 2ch+1) = k-tiles (ch, ch+4)
                        # emission T T copy copy L L: the copy for chunk
                        # 2q hides under transpose 2q+1 on the PE.
                        ptrs = {}
                        for ch in (2 * q, 2 * q + 1):
                            ptrh = psum_t.tile([128, 128], bf16, tag="ptr")
                            nc.tensor.transpose(
                                ptrh, hn[:, ch * 128:(ch + 1) * 128], ident)
                            ptrs[ch] = ptrh
                        for ch in (2 * q, 2 * q + 1):
                            _emit_logits(nc, ptrs, hT_new, pl, wout, ch)

                    # ---------------- softmax (folded) ----------------
                    eu = work.tile([128, OF], f32, tag="eu")
                    ssum = work.tile([128, 1], f32, tag="ssum")
                    nc.scalar.activation(out=eu, in_=pl, func=Exp,
                                         accum_out=ssum)
                    stmp = work.tile([B, 1], f32, tag="stmp")
                    nc.vector.tensor_copy(out=stmp, in_=ssum[B:128])
                    sden = work.tile([B, 1], f32, tag="sden")
                    nc.vector.tensor_add(out=sden, in0=ssum[0:B],
                                         in1=stmp)
                    sinv = work.tile([128, 1], f32, tag="sinv")
                    nc.vector.reciprocal(out=sinv[0:B], in_=sden)
                    nc.vector.reciprocal(out=sinv[B:128], in_=sden)
                    y = ys.tile([128, OF], f32, tag="y")
                    nc.scalar.mul(out=y[:, 0:128], in_=eu[:, 0:128],
                                  mul=sinv)
                    nc.scalar.mul(out=y[:, 128:OF], in_=eu[:, 128:OF],
                                  mul=sinv)
                    trow = (T - 1 - t) % T
                    nc.sync.dma_start(out=out_d[trow, :, 0:OF],
                                      in_=y[0:B, :])
                    nc.sync.dma_start(out=out_d[trow, :, OF:O],
                                      in_=y[B:128, :])
                    if t < t_live - 1 or TIMING_REPS:
                        ybf = ys.tile([128, OF], bf16, tag="ybf")
                        nc.scalar.mul(out=ybf, in_=eu, mul=sinv)
                        pend_ybf = ybf

                    c_prev = c_new
                    hT_prev = hT_new

    nc.compile()
    return nc


def _host_prep(h0, c0, W_ih, W_hh, b_ih, b_hh, W_out, b_out):
    """Build per-core input maps (host-side layout transforms)."""
    f32 = np.float32
    h0 = np.asarray(h0, f32).reshape(NCORES * B, H)
    c0 = np.asarray(c0, f32).reshape(NCORES * B, H)
    W_ih = np.asarray(W_ih, f32)
    W_hh = np.asarray(W_hh, f32)
    W_out = np.asarray(W_out, f32)
    b_tot = np.asarray(b_ih, f32) + np.asarray(b_hh, f32)
    b_out = np.asarray(b_out, f32)

    # permute gate order (i, f, g, o) -> (i, f, o, g)
    perm = np.r_[0:H, H:2 * H, 3 * H:4 * H, 2 * H:3 * H]
    Wih_p = W_ih[perm]          # [G, O]
    Whh_p = W_hh[perm]          # [G, H]
    b_p = b_tot[perm]           # [G]

    # weight layout: [p, kpos, gate, h-half, h-col] flattened to [128, K*G],
    # with k-tiles permuted so transposed activations land in order.
    WihT_aug = Wih_p.T + b_p[None, :]           # [O, G]
    wih_host = np.ascontiguousarray(
        WihT_aug.reshape(KX, 128, 4, 2, HF)[KXORDER].transpose(1, 0, 2, 3, 4)
    ).reshape(128, KX * G).astype(_BF16)
    whh_host = np.ascontiguousarray(
        Whh_p.T.reshape(KH, 128, 4, 2, HF)[KORDER].transpose(1, 0, 2, 3, 4)
    ).reshape(128, KH * G).astype(_BF16)
    # wout blocks: [kpos, O-half, O-col]
    wout_host = np.ascontiguousarray(
        W_out.T.reshape(KH, 128, O)[KORDER].transpose(1, 0, 2)
    ).reshape(128, KH * O).astype(_BF16)
    biasrow = b_p[None, :].astype(_BF16)        # [1, (gate, half, col)]
    boutrow = b_out[None, :].astype(_BF16)
    onesrow = np.ones((1, B), _BF16)
    ident = np.eye(128).astype(_BF16)

    in_maps = []
    for i in range(NCORES):
        sl = slice(i * B, (i + 1) * B)
        h0s = h0[sl]                                # [B, H]
        h0t = np.ascontiguousarray(
            h0s.reshape(B, KH, 128).transpose(1, 2, 0)[KORDER]
        ).reshape(KH, 128, B).transpose(1, 0, 2).reshape(128, KH * B)
        c0f = np.ascontiguousarray(
            c0[sl].reshape(B, 2, HF).transpose(1, 0, 2)).reshape(128, HF)
        in_maps.append({
            "wih": wih_host, "whh": whh_host, "wout": wout_host,
            "biasrow": biasrow, "boutrow": boutrow, "onesrow": onesrow,
            "ident": ident,
            "h0t": np.ascontiguousarray(h0t).astype(_BF16),
            "c0": c0f,
        })
    return in_maps


def kernel(h0, c0, W_ih, W_hh, b_ih, b_hh, W_out, b_out, out_len):
    from concourse.bass_utils import run_bass_kernel_spmd

    assert int(out_len) == T
    if "nc" not in _cache:
        _cache["nc"] = _build()
    nc = _cache["nc"]
    in_maps = _host_prep(h0, c0, W_ih, W_hh, b_ih, b_hh, W_out, b_out)
    res = run_bass_kernel_spmd(nc, in_maps, core_ids=list(range(NCORES)))
    full = np.empty((T, NCORES * B, O), np.float32)
    for i in range(NCORES):
        full[:, i * B:(i + 1) * B, :] = res.results[i]["out"]
    return full


# revision 15
# speedup vs baseline: 1.1530x; 1.0706x over previous
"""Bass/Trainium2 kernel for nn_DecoderRNN: feedback LSTM decoder.

Math per step (PyTorch LSTMCell, gates (i,f,g,o)):
    gates = x @ W_ih.T + b_ih + h @ W_hh.T + b_hh     x = prev softmax output
    c' = sig(f)*c + sig(i)*tanh(g);  h' = sig(o)*tanh(c')
    y  = softmax(h' @ W_out.T + b_out);  x_next = y
Output is time-reversed: out[T-1-t] = y_t.

Sharding: data-parallel over batch across 8 cores (B=512 -> 64/core),
weights replicated, recurrence local per core.

Device-side design (per core, B=64):
- "H-folded" layout: every per-gate [B, 1024] tensor is stored as
  [128, 512] with partition p = j*64 + b (j = h-half).  This fills all
  128 partitions/PE columns even though the per-core batch is only 64,
  and keeps every elementwise op lane-local.
- gates are computed per-gate (chunks of 512 h-features x 2 halves):
  stationary = xT/hT k-tiles [128, 64]; the two h-halves run as
  concurrent column-group matmuls (tile positions (0,0) / (0,64))
  accumulating into one PSUM bank.  (HW-measured: a pair costs ~257ns
  vs 241ns for one serial MM -> ~1.85x concurrency.)
- logits are ALSO folded: pl [128, 256] with p = jO*64 + b (jO =
  O-half), so each hT k-tile drives a col-group PAIR of N=256 streams.
  Softmax then needs a cross-half sum: ssum[0:64] + ssum[64:128].
- h'/y stay folded; transposes back to [feature, batch] are full
  [128,128] PE transposes (4 for h, 2 for y vs 8+4 thin ones before).
  One [128,128] transpose of folded data yields TWO k-tiles (h-halves
  interleave), so hT/xT column order is permuted: korder [0,4,1,5,...]
  (h) / [0,2,1,3] (x); weight layouts are permuted to match host-side.
- gate order permuted to (i, f, o, g); sigmoid computed as
  0.5 + 0.5*tanh(x/2) so only the exp_and_others ACT table set is used
  (tanh + exp; no table swaps).
- b_ih+b_hh folded into W_ih.T rows (softmax x sums to exactly 1, so
  adding b to every row of W_ih.T adds b*sum(x) = b).  Step 0 has x=0,
  so its bias comes from K=1 ones-matmuls against a bias row instead.
- b_out added via a K=1 ones-matmul pair into the logits PSUM.
"""

import numpy as np
import ml_dtypes

B = 64          # batch per core
H = 1024
HF = 512        # folded h-half size
O = 512
OF = 256        # folded O-half size
G = 4 * H       # 4096
T = 256
KH = H // 128   # 8 h k-tiles
KX = O // 128   # 4 x k-tiles
NCORES = 8

KORDER = [0, 4, 1, 5, 2, 6, 3, 7]   # hT column-block -> h k-tile
KXORDER = [0, 2, 1, 3]              # xT column-block -> x k-tile

_BF16 = ml_dtypes.bfloat16

_cache = {}

# Number of steps actually emitted (out buffer stays [T, B, O]); test
# harnesses may lower this to build a transfer-identical baseline module.
T_LIVE = None
# When set (int R), wraps the whole step loop in a hardware For_i loop so
# the body executes R times — used to measure per-step time above host
# noise.  Output values are garbage after the first iteration.
TIMING_REPS = None


def _build():
    import concourse.bass as bass
    import concourse.tile as tile
    from concourse import bacc, mybir

    f32 = mybir.dt.float32
    bf16 = mybir.dt.bfloat16
    Tanh = mybir.ActivationFunctionType.Tanh
    Exp = mybir.ActivationFunctionType.Exp

    nc = bacc.Bacc("TRN2", target_bir_lowering=False, debug=False,
                   num_devices=NCORES)

    # ---- DRAM I/O ----
    # wih: [128, KX*4*2*512]  (kx-block, gate, h-half, h-col), bias folded
    # whh: [128, KH*4*2*512]  (k-block, gate, h-half, h-col)
    # wout: [128, KH*2*256]   (k-block, O-half, O-col)
    wih_d = nc.dram_tensor("wih", [128, KX * G], bf16, kind="ExternalInput")
    whh_d = nc.dram_tensor("whh", [128, KH * G], bf16, kind="ExternalInput")
    wout_d = nc.dram_tensor("wout", [128, KH * O], bf16, kind="ExternalInput")
    biasrow_d = nc.dram_tensor("biasrow", [1, G], bf16, kind="ExternalInput")
    boutrow_d = nc.dram_tensor("boutrow", [1, O], bf16, kind="ExternalInput")
    onesrow_d = nc.dram_tensor("onesrow", [1, B], bf16, kind="ExternalInput")
    ident_d = nc.dram_tensor("ident", [128, 128], bf16, kind="ExternalInput")
    h0t_d = nc.dram_tensor("h0t", [128, KH * B], bf16, kind="ExternalInput")
    c0_d = nc.dram_tensor("c0", [128, HF], f32, kind="ExternalInput")
    out_d = nc.dram_tensor("out", [T, B, O], f32, kind="ExternalOutput")

    with tile.TileContext(nc) as tc:
        with (
            tc.tile_pool(name="consts", bufs=1) as consts,
            tc.tile_pool(name="state_c", bufs=2) as state_c,
            tc.tile_pool(name="state_ht", bufs=2) as state_ht,
            tc.tile_pool(name="state_xt", bufs=2) as state_xt,
            tc.tile_pool(name="work", bufs=2) as work,
            tc.tile_pool(name="ys", bufs=3) as ys,
            tc.tile_pool(name="psum_g", bufs=4, space="PSUM") as psum_g,
            tc.tile_pool(name="psum_l", bufs=1, space="PSUM") as psum_l,
            tc.tile_pool(name="psum_t", bufs=3, space="PSUM") as psum_t,
        ):
            # ---- load constants ----
            wih = consts.tile([128, KX * G], bf16)
            nc.sync.dma_start(out=wih, in_=wih_d[:, :])
            whh = consts.tile([128, KH * G], bf16)
            nc.sync.dma_start(out=whh, in_=whh_d[:, :])
            wout = consts.tile([128, KH * O], bf16)
            nc.sync.dma_start(out=wout, in_=wout_d[:, :])
            biasrow = consts.tile([1, G], bf16)
            nc.sync.dma_start(out=biasrow, in_=biasrow_d[:, :])
            boutrow = consts.tile([1, O], bf16)
            nc.sync.dma_start(out=boutrow, in_=boutrow_d[:, :])
            onesrow = consts.tile([1, B], bf16)
            nc.sync.dma_start(out=onesrow, in_=onesrow_d[:, :])
            ident = consts.tile([128, 128], bf16)
            nc.sync.dma_start(out=ident, in_=ident_d[:, :])

            c_prev = state_c.tile([128, HF], f32, tag="c")
            nc.sync.dma_start(out=c_prev, in_=c0_d[:, :])
            hT_prev = state_ht.tile([128, KH * B], bf16, tag="ht")
            nc.sync.dma_start(out=hT_prev, in_=h0t_d[:, :])
            xT_prev = None

            def wslice(w, i, g, j):
                # weight block for k-position i, gate g, h-half j: [128, 512]
                base = ((i * 4 + g) * 2 + j) * HF
                return w[:, base:base + HF]

            def _emit_logits(nc, ptrs, hT_new, pl, wout, ch):
                # copy transpose-psum chunk ch into hT, then its two
                # k-positions' logits col-pair matmuls
                nc.vector.tensor_copy(
                    out=hT_new[:, 2 * ch * B:(2 * ch + 2) * B],
                    in_=ptrs[ch])
                for i in (2 * ch, 2 * ch + 1):
                    lastk = i == KH - 1
                    nc.tensor.matmul(
                        pl[0:B, :], hT_new[:, i * B:(i + 1) * B],
                        wout[:, i * O:i * O + OF],
                        start=False, stop=lastk,
                        skip_group_check=True)
                    nc.tensor.matmul(
                        pl[B:128, :], hT_new[:, i * B:(i + 1) * B],
                        wout[:, i * O + OF:(i + 1) * O],
                        start=False, stop=lastk,
                        skip_group_check=True)

            t_live = T if T_LIVE is None else T_LIVE
            from contextlib import nullcontext
            loop_ctx = (tc.For_i(0, int(TIMING_REPS), 1)
                        if TIMING_REPS else nullcontext())
            with loop_ctx:
                pend_ybf = None
                for t in range(t_live):
                    # ---------------- gates: h-part (all 4 chunks) ------
                    # emission order (o, i, g, f): f last => only f's
                    # act->sig->c->h chain is on the step-boundary critical
                    # path; o/i/g activations and u2 run during f's matmuls.
                    tg = work.tile([128, 4, HF], bf16, tag="tg")
                    sg = work.tile([128, 3, HF], bf16, tag="sg")
                    pgs = {}
                    for g in (0, 3, 1, 2):  # emission order: i, g, f, o
                        pg = psum_g.tile([128, HF], f32, tag="pg")
                        pgs[g] = pg
                        for i in range(KH):
                            nc.tensor.matmul(pg[0:B, :],
                                             hT_prev[:, i * B:(i + 1) * B],
                                             wslice(whh, i, g, 0),
                                             start=(i == 0), stop=False,
                                             skip_group_check=True)
                            nc.tensor.matmul(pg[B:128, :],
                                             hT_prev[:, i * B:(i + 1) * B],
                                             wslice(whh, i, g, 1),
                                             start=(i == 0), stop=False,
                                             skip_group_check=True)

                    # ---- previous step's y_bf -> xT transposes ----
                    if pend_ybf is not None:
                        ybf = pend_ybf
                        xT_new = state_xt.tile([128, KX * B], bf16,
                                               tag="xt")
                        for cx in range(2):
                            ptry = psum_t.tile([128, 128], bf16, tag="ptr")
                            nc.tensor.transpose(
                                ptry,
                                ybf[:, cx * 128:(cx + 1) * 128], ident)
                            nc.vector.tensor_copy(
                                out=xT_new[:, cx * 128:(cx + 1) * 128],
                                in_=ptry)
                        xT_prev = xT_new
                        pend_ybf = None

                    # ---------------- gates: x-part + activations -------
                    # o is last: the c/tanh chain (needs i,g,f) is emitted
                    # right after f and runs during o's matmuls; only
                    # act(o) -> sig(o) -> hn remains on the critical path,
                    # and that tail is split by column halves.
                    u1 = work.tile([128, HF], f32, tag="u1")
                    u2 = work.tile([128, HF], bf16, tag="u2")
                    c_new = state_c.tile([128, HF], f32, tag="c")
                    th = work.tile([128, HF], bf16, tag="th")
                    hn = work.tile([128, HF], bf16, tag="hn")
                    HQ = HF // 2
                    for g in (0, 3, 1, 2):
                        pg = pgs[g]
                        tail = g == 2   # o: last gate, on critical path
                        if t == 0:  # bias via K=1 ones-matmul
                            b0 = (g * 2) * HF
                            nc.tensor.matmul(pg[0:B, :], onesrow,
                                             biasrow[:, b0:b0 + HF],
                                             start=False, stop=True,
                                             skip_group_check=True)
                            nc.tensor.matmul(pg[B:128, :], onesrow,
                                             biasrow[:, b0 + HF:b0 + 2 * HF],
                                             start=False, stop=True,
                                             skip_group_check=True)
                        else:
                            for i in range(KX):
                                last = i == KX - 1
                                if tail and last:
                                    # split the final accumulating pair by
                                    # column halves so act(o) starts early
                                    for jj in range(2):
                                        hs = slice(jj * HQ, (jj + 1) * HQ)
                                        nc.tensor.matmul(
                                            pg[0:B, hs],
                                            xT_prev[:, i * B:(i + 1) * B],
                                            wslice(wih, i, g, 0)[:, hs],
                                            start=False, stop=True,
                                            skip_group_check=True)
                                        nc.tensor.matmul(
                                            pg[B:128, hs],
                                            xT_prev[:, i * B:(i + 1) * B],
                                            wslice(wih, i, g, 1)[:, hs],
                                            start=False, stop=True,
                                            skip_group_check=True)
                                    continue
                                nc.tensor.matmul(
                                    pg[0:B, :],
                                    xT_prev[:, i * B:(i + 1) * B],
                                    wslice(wih, i, g, 0),
                                    start=False, stop=last,
                                    skip_group_check=True)
                                nc.tensor.matmul(
                                    pg[B:128, :],
                                    xT_prev[:, i * B:(i + 1) * B],
                                    wslice(wih, i, g, 1),
                                    start=False, stop=last,
                                    skip_group_check=True)
                        # tanh for this gate (x/2 for i,f,o; g==3 plain)
                        if tail:
                            for q in range(2):
                                cs = slice(q * HQ, (q + 1) * HQ)
                                nc.scalar.activation(
                                    out=tg[:, g, cs], in_=pg[:, cs],
                                    func=Tanh, scale=0.5)
                            for q in range(2):
                                cs = slice(q * HQ, (q + 1) * HQ)
                                nc.vector.tensor_scalar(
                                    out=sg[:, g, cs], in0=tg[:, g, cs],
                                    scalar1=0.5, scalar2=0.5,
                                    op0=mybir.AluOpType.mult,
                                    op1=mybir.AluOpType.add)
                        else:
                            nc.scalar.activation(
                                out=tg[:, g, :], in_=pg, func=Tanh,
                                scale=0.5 if g != 3 else 1.0)
                            if g != 3:  # sigmoid:  s = 0.5*tanh + 0.5
                                nc.vector.tensor_scalar(
                                    out=sg[:, g, :], in0=tg[:, g, :],
                                    scalar1=0.5, scalar2=0.5,
                                    op0=mybir.AluOpType.mult,
                                    op1=mybir.AluOpType.add)
                        if g == 3:
                            nc.gpsimd.tensor_mul(out=u2, in0=sg[:, 0, :],
                                                 in1=tg[:, 3, :])
                        if g == 1:
                            # c / tanh(c) off-chain, during o's matmuls
                            for q in range(2):
                                cs = slice(q * HQ, (q + 1) * HQ)
                                nc.vector.tensor_mul(out=u1[:, cs],
                                                     in0=sg[:, 1, cs],
                                                     in1=c_prev[:, cs])
                                nc.vector.tensor_add(out=c_new[:, cs],
                                                     in0=u1[:, cs],
                                                     in1=u2[:, cs])
                                nc.scalar.activation(out=th[:, cs],
                                                     in_=c_new[:, cs],
                                                     func=Tanh)

                    # logits psum: bias pair opens the accumulation
                    pl = psum_l.tile([128, OF], f32, tag="pl")
                    nc.tensor.matmul(pl[0:B, :], onesrow, boutrow[:, 0:OF],
                                     start=True, stop=False,
                                     skip_group_check=True)
                    nc.tensor.matmul(pl[B:128, :], onesrow,
                                     boutrow[:, OF:O],
                                     start=True, stop=False,
                                     skip_group_check=True)

                    hT_new = state_ht.tile([128, KH * B], bf16, tag="ht")
                    for q in range(2):
                        cs = slice(q * HQ, (q + 1) * HQ)
                        nc.vector.tensor_mul(out=hn[:, cs],
                                             in0=sg[:, 2, cs],
                                             in1=th[:, cs])
                        # two [128,128] transposes per half; each yields
                        # hT column-blocks (2ch, 2ch+1) = k-tiles (ch, ch+4)
                        # emission T T copy copy L L: the copy for chunk
                        # 2q hides under transpose 2q+1 on the PE.
                        ptrs = {}
                        for ch in (2 * q, 2 * q + 1):
                            ptrh = psum_t.tile([128, 128], bf16, tag="ptr")
                            nc.tensor.transpose(
                                ptrh, hn[:, ch * 128:(ch + 1) * 128], ident)
                            ptrs[ch] = ptrh
                        for ch in (2 * q, 2 * q + 1):
                            _emit_logits(nc, ptrs, hT_new, pl, wout, ch)

                    # ---------------- softmax (folded) ----------------
                    eu = work.tile([128, OF], f32, tag="eu")
                    ssum = work.tile([128, 1], f32, tag="ssum")
                    nc.scalar.activation(out=eu, in_=pl, func=Exp,
                                         accum_out=ssum)
                    stmp = work.tile([B, 1], f32, tag="stmp")
                    nc.vector.tensor_copy(out=stmp, in_=ssum[B:128])
                    sden = work.tile([B, 1], f32, tag="sden")
                    nc.vector.tensor_add(out=sden, in0=ssum[0:B],
                                         in1=stmp)
                    sinv = work.tile([128, 1], f32, tag="sinv")
                    nc.vector.reciprocal(out=sinv[0:B], in_=sden)
                    nc.vector.reciprocal(out=sinv[B:128], in_=sden)
                    y = ys.tile([128, OF], f32, tag="y")
                    nc.scalar.mul(out=y[:, 0:128], in_=eu[:, 0:128],
                                  mul=sinv)
                    nc.scalar.mul(out=y[:, 128:OF], in_=eu[:, 128:OF],
                                  mul=sinv)
                    trow = (T - 1 - t) % T
                    nc.sync.dma_start(out=out_d[trow, :, 0:OF],
                                      in_=y[0:B, :])
                    nc.sync.dma_start(out=out_d[trow, :, OF:O],
                                      in_=y[B:128, :])
                    if t < t_live - 1 or TIMING_REPS:
                        ybf = ys.tile([128, OF], bf16, tag="ybf")
                        nc.scalar.mul(out=ybf, in_=eu, mul=sinv)
                        pend_ybf = ybf

                    c_prev = c_new
                    hT_prev = hT_new

    nc.compile()
    return nc


def _host_prep(h0, c0, W_ih, W_hh, b_ih, b_hh, W_out, b_out):
    """Build per-core input maps (host-side layout transforms)."""
    f32 = np.float32
    h0 = np.asarray(h0, f32).reshape(NCORES * B, H)
    c0 = np.asarray(c0, f32).reshape(NCORES * B, H)
    W_ih = np.asarray(W_ih, f32)
    W_hh = np.asarray(W_hh, f32)
    W_out = np.asarray(W_out, f32)
    b_tot = np.asarray(b_ih, f32) + np.asarray(b_hh, f32)
    b_out = np.asarray(b_out, f32)

    # permute gate order (i, f, g, o) -> (i, f, o, g)
    perm = np.r_[0:H, H:2 * H, 3 * H:4 * H, 2 * H:3 * H]
    Wih_p = W_ih[perm]          # [G, O]
    Whh_p = W_hh[perm]          # [G, H]
    b_p = b_tot[perm]           # [G]

    # weight layout: [p, kpos, gate, h-half, h-col] flattened to [128, K*G],
    # with k-tiles permuted so transposed activations land in order.
    WihT_aug = Wih_p.T + b_p[None, :]           # [O, G]
    wih_host = np.ascontiguousarray(
        WihT_aug.reshape(KX, 128, 4, 2, HF)[KXORDER].transpose(1, 0, 2, 3, 4)
    ).reshape(128, KX * G).astype(_BF16)
    whh_host = np.ascontiguousarray(
        Whh_p.T.reshape(KH, 128, 4, 2, HF)[KORDER].transpose(1, 0, 2, 3, 4)
    ).reshape(128, KH * G).astype(_BF16)
    # wout blocks: [kpos, O-half, O-col]
    wout_host = np.ascontiguousarray(
        W_out.T.reshape(KH, 128, O)[KORDER].transpose(1, 0, 2)
    ).reshape(128, KH * O).astype(_BF16)
    biasrow = b_p[None, :].astype(_BF16)        # [1, (gate, half, col)]
    boutrow = b_out[None, :].astype(_BF16)
    onesrow = np.ones((1, B), _BF16)
    ident = np.eye(128).astype(_BF16)

    in_maps = []
    for i in range(NCORES):
        sl = slice(i * B, (i + 1) * B)
        h0s = h0[sl]                                # [B, H]
        h0t = np.ascontiguousarray(
            h0s.reshape(B, KH, 128).transpose(1, 2, 0)[KORDER]
        ).reshape(KH, 128, B).transpose(1, 0, 2).reshape(128, KH * B)
        c0f = np.ascontiguousarray(
            c0[sl].reshape(B, 2, HF).transpose(1, 0, 2)).reshape(128, HF)
        in_maps.append({
            "wih": wih_host, "whh": whh_host, "wout": wout_host,
            "biasrow": biasrow, "boutrow": boutrow, "onesrow": onesrow,
            "ident": ident,
            "h0t": np.ascontiguousarray(h0t).astype(_BF16),
            "c0": c0f,
        })
    return in_maps


def kernel(h0, c0, W_ih, W_hh, b_ih, b_hh, W_out, b_out, out_len):
    from concourse.bass_utils import run_bass_kernel_spmd

    assert int(out_len) == T
    if "nc" not in _cache:
        _cache["nc"] = _build()
    nc = _cache["nc"]
    in_maps = _host_prep(h0, c0, W_ih, W_hh, b_ih, b_hh, W_out, b_out)
    res = run_bass_kernel_spmd(nc, in_maps, core_ids=list(range(NCORES)))
    full = np.empty((T, NCORES * B, O), np.float32)
    for i in range(NCORES):
        full[:, i * B:(i + 1) * B, :] = res.results[i]["out"]
    return full


# revision 19
# speedup vs baseline: 1.1778x; 1.0215x over previous
"""Bass/Trainium2 kernel for nn_DecoderRNN: feedback LSTM decoder.

Math per step (PyTorch LSTMCell, gates (i,f,g,o)):
    gates = x @ W_ih.T + b_ih + h @ W_hh.T + b_hh     x = prev softmax output
    c' = sig(f)*c + sig(i)*tanh(g);  h' = sig(o)*tanh(c')
    y  = softmax(h' @ W_out.T + b_out);  x_next = y
Output is time-reversed: out[T-1-t] = y_t.

Sharding: data-parallel over batch across 8 cores (B=512 -> 64/core),
weights replicated, recurrence local per core.

Device-side design (per core, B=64):
- "H-folded" layout: every per-gate [B, 1024] tensor is stored as
  [128, 512] with partition p = j*64 + b (j = h-half).  This fills all
  128 partitions/PE columns even though the per-core batch is only 64,
  and keeps every elementwise op lane-local.
- gates are computed per-gate (chunks of 512 h-features x 2 halves):
  stationary = xT/hT k-tiles [128, 64]; the two h-halves run as
  concurrent column-group matmuls (tile positions (0,0) / (0,64))
  accumulating into one PSUM bank.  (HW-measured: a pair costs ~257ns
  vs 241ns for one serial MM -> ~1.85x concurrency.)
- logits are ALSO folded: pl [128, 256] with p = jO*64 + b (jO =
  O-half), so each hT k-tile drives a col-group PAIR of N=256 streams.
  Softmax then needs a cross-half sum: ssum[0:64] + ssum[64:128].
- h'/y stay folded; transposes back to [feature, batch] are full
  [128,128] PE transposes (4 for h, 2 for y vs 8+4 thin ones before).
  One [128,128] transpose of folded data yields TWO k-tiles (h-halves
  interleave), so hT/xT column order is permuted: korder [0,4,1,5,...]
  (h) / [0,2,1,3] (x); weight layouts are permuted to match host-side.
- gate order permuted to (i, f, o, g); sigmoid computed as
  0.5 + 0.5*tanh(x/2) so only the exp_and_others ACT table set is used
  (tanh + exp; no table swaps).
- b_ih+b_hh folded into W_ih.T rows (softmax x sums to exactly 1, so
  adding b to every row of W_ih.T adds b*sum(x) = b).  Step 0 has x=0,
  so its bias comes from K=1 ones-matmuls against a bias row instead.
- b_out added via a K=1 ones-matmul pair into the logits PSUM.
"""

import numpy as np
import ml_dtypes

B = 64          # batch per core
H = 1024
HF = 512        # folded h-half size
O = 512
OF = 256        # folded O-half size
G = 4 * H       # 4096
T = 256
KH = H // 128   # 8 h k-tiles
KX = O // 128   # 4 x k-tiles
NCORES = 8

KORDER = [0, 4, 1, 5, 2, 6, 3, 7]   # hT column-block -> h k-tile
KXORDER = [0, 2, 1, 3]              # xT column-block -> x k-tile

_BF16 = ml_dtypes.bfloat16

_cache = {}

# Number of steps actually emitted (out buffer stays [T, B, O]); test
# harnesses may lower this to build a transfer-identical baseline module.
T_LIVE = None
# When set (int R), wraps the whole step loop in a hardware For_i loop so
# the body executes R times — used to measure per-step time above host
# noise.  Output values are garbage after the first iteration.
TIMING_REPS = None


def _build():
    import concourse.bass as bass
    import concourse.tile as tile
    from concourse import bacc, mybir

    f32 = mybir.dt.float32
    bf16 = mybir.dt.bfloat16
    Tanh = mybir.ActivationFunctionType.Tanh
    Exp = mybir.ActivationFunctionType.Exp

    nc = bacc.Bacc("TRN2", target_bir_lowering=False, debug=False,
                   num_devices=NCORES)

    # ---- DRAM I/O ----
    # wih: [128, KX*4*2*512]  (kx-block, gate, h-half, h-col), bias folded
    # whh: [128, KH*4*2*512]  (k-block, gate, h-half, h-col)
    # wout: [128, KH*2*256]   (k-block, O-half, O-col)
    wih_d = nc.dram_tensor("wih", [128, KX * G], bf16, kind="ExternalInput")
    whh_d = nc.dram_tensor("whh", [128, KH * G], bf16, kind="ExternalInput")
    wout_d = nc.dram_tensor("wout", [128, KH * O], bf16, kind="ExternalInput")
    biasrow_d = nc.dram_tensor("biasrow", [1, G], bf16, kind="ExternalInput")
    boutrow_d = nc.dram_tensor("boutrow", [1, O], bf16, kind="ExternalInput")
    onesrow_d = nc.dram_tensor("onesrow", [1, B], bf16, kind="ExternalInput")
    ident_d = nc.dram_tensor("ident", [128, 128], bf16, kind="ExternalInput")
    h0t_d = nc.dram_tensor("h0t", [128, KH * B], bf16, kind="ExternalInput")
    c0_d = nc.dram_tensor("c0", [128, HF], f32, kind="ExternalInput")
    out_d = nc.dram_tensor("out", [T, B, O], f32, kind="ExternalOutput")

    with tile.TileContext(nc) as tc:
        with (
            tc.tile_pool(name="consts", bufs=1) as consts,
            tc.tile_pool(name="state_c", bufs=2) as state_c,
            tc.tile_pool(name="state_ht", bufs=2) as state_ht,
            tc.tile_pool(name="state_xt", bufs=2) as state_xt,
            tc.tile_pool(name="work", bufs=2) as work,
            tc.tile_pool(name="ys", bufs=3) as ys,
            tc.tile_pool(name="psum_g", bufs=4, space="PSUM") as psum_g,
            tc.tile_pool(name="psum_l", bufs=1, space="PSUM") as psum_l,
            tc.tile_pool(name="psum_t", bufs=3, space="PSUM") as psum_t,
        ):
            # ---- load constants ----
            wih = consts.tile([128, KX * G], bf16)
            nc.sync.dma_start(out=wih, in_=wih_d[:, :])
            whh = consts.tile([128, KH * G], bf16)
            nc.sync.dma_start(out=whh, in_=whh_d[:, :])
            wout = consts.tile([128, KH * O], bf16)
            nc.sync.dma_start(out=wout, in_=wout_d[:, :])
            biasrow = consts.tile([1, G], bf16)
            nc.sync.dma_start(out=biasrow, in_=biasrow_d[:, :])
            boutrow = consts.tile([1, O], bf16)
            nc.sync.dma_start(out=boutrow, in_=boutrow_d[:, :])
            onesrow = consts.tile([1, B], bf16)
            nc.sync.dma_start(out=onesrow, in_=onesrow_d[:, :])
            ident = consts.tile([128, 128], bf16)
            nc.sync.dma_start(out=ident, in_=ident_d[:, :])

            c_prev = state_c.tile([128, HF], f32, tag="c")
            nc.sync.dma_start(out=c_prev, in_=c0_d[:, :])
            hT_prev = state_ht.tile([128, KH * B], bf16, tag="ht")
            nc.sync.dma_start(out=hT_prev, in_=h0t_d[:, :])
            xT_prev = None

            def wslice(w, i, g, j):
                # weight block for k-position i, gate g, h-half j: [128, 512]
                base = ((i * 4 + g) * 2 + j) * HF
                return w[:, base:base + HF]

            def _emit_logits(nc, ptrs, hT_new, pl, wout, ch):
                # copy transpose-psum chunk ch into hT, then its two
                # k-positions' logits col-pair matmuls
                nc.vector.tensor_copy(
                    out=hT_new[:, 2 * ch * B:(2 * ch + 2) * B],
                    in_=ptrs[ch])
                for i in (2 * ch, 2 * ch + 1):
                    lastk = i == KH - 1
                    nc.tensor.matmul(
                        pl[0:B, :], hT_new[:, i * B:(i + 1) * B],
                        wout[:, i * O:i * O + OF],
                        start=False, stop=lastk,
                        skip_group_check=True)
                    nc.tensor.matmul(
                        pl[B:128, :], hT_new[:, i * B:(i + 1) * B],
                        wout[:, i * O + OF:(i + 1) * O],
                        start=False, stop=lastk,
                        skip_group_check=True)

            t_live = T if T_LIVE is None else T_LIVE
            from contextlib import nullcontext
            loop_ctx = (tc.For_i(0, int(TIMING_REPS), 1)
                        if TIMING_REPS else nullcontext())
            with loop_ctx:
                pend_ybf = None
                for t in range(t_live):
                    # ---------------- gates: h-part (all 4 chunks) ------
                    # emission order (o, i, g, f): f last => only f's
                    # act->sig->c->h chain is on the step-boundary critical
                    # path; o/i/g activations and u2 run during f's matmuls.
                    tg = work.tile([128, 4, HF], bf16, tag="tg")
                    sg = work.tile([128, 3, HF], bf16, tag="sg")
                    pgs = {}
                    for g in (0, 3, 1, 2):
                        pgs[g] = psum_g.tile([128, HF], f32, tag="pg",
                                             name=f"pg{g}")
                    # gate-pair blocking: both gates of a pair stream against
                    # the same stationary hT k-tile, so LDWEIGHTS per col
                    # group is amortized 2x (HW-measured ~15ns/pair saving)
                    for gp in ((0, 3), (1, 2)):
                        for i in range(KH):
                            for g in gp:
                                pg = pgs[g]
                                nc.tensor.matmul(
                                    pg[0:B, :],
                                    hT_prev[:, i * B:(i + 1) * B],
                                    wslice(whh, i, g, 0),
                                    start=(i == 0), stop=False,
                                    skip_group_check=True)
                                nc.tensor.matmul(
                                    pg[B:128, :],
                                    hT_prev[:, i * B:(i + 1) * B],
                                    wslice(whh, i, g, 1),
                                    start=(i == 0), stop=False,
                                    skip_group_check=True)

                    # ---- previous step's y_bf -> xT transposes ----
                    if pend_ybf is not None:
                        ybf = pend_ybf
                        xT_new = state_xt.tile([128, KX * B], bf16,
                                               tag="xt")
                        for cx in range(2):
                            ptry = psum_t.tile([128, 128], bf16, tag="ptr")
                            nc.tensor.transpose(
                                ptry,
                                ybf[:, cx * 128:(cx + 1) * 128], ident)
                            nc.vector.tensor_copy(
                                out=xT_new[:, cx * 128:(cx + 1) * 128],
                                in_=ptry)
                        xT_prev = xT_new
                        pend_ybf = None

                    # ---------------- gates: x-part + activations -------
                    for g in (0, 3, 1, 2):
                        pg = pgs[g]
                        if t == 0:  # bias via K=1 ones-matmul
                            b0 = (g * 2) * HF
                            nc.tensor.matmul(pg[0:B, :], onesrow,
                                             biasrow[:, b0:b0 + HF],
                                             start=False, stop=True,
                                             skip_group_check=True)
                            nc.tensor.matmul(pg[B:128, :], onesrow,
                                             biasrow[:, b0 + HF:b0 + 2 * HF],
                                             start=False, stop=True,
                                             skip_group_check=True)
                        else:
                            for i in range(KX):
                                last = i == KX - 1
                                nc.tensor.matmul(
                                    pg[0:B, :],
                                    xT_prev[:, i * B:(i + 1) * B],
                                    wslice(wih, i, g, 0),
                                    start=False, stop=last,
                                    skip_group_check=True)
                                nc.tensor.matmul(
                                    pg[B:128, :],
                                    xT_prev[:, i * B:(i + 1) * B],
                                    wslice(wih, i, g, 1),
                                    start=False, stop=last,
                                    skip_group_check=True)
                        # tanh for this gate (x/2 for i,f,o)
                        nc.scalar.activation(
                            out=tg[:, g, :], in_=pg, func=Tanh,
                            scale=0.5 if g < 3 else 1.0)
                        if g < 3:  # sigmoid:  s = 0.5*tanh + 0.5
                            nc.vector.tensor_scalar(
                                out=sg[:, g, :], in0=tg[:, g, :],
                                scalar1=0.5, scalar2=0.5,
                                op0=mybir.AluOpType.mult,
                                op1=mybir.AluOpType.add)

                    # ------- c / h update, split in column halves -------
                    u2 = work.tile([128, HF], bf16, tag="u2")
                    nc.gpsimd.tensor_mul(out=u2, in0=sg[:, 0, :],
                                         in1=tg[:, 3, :])
                    u1 = work.tile([128, HF], f32, tag="u1")
                    c_new = state_c.tile([128, HF], f32, tag="c")
                    th = work.tile([128, HF], bf16, tag="th")
                    hn = work.tile([128, HF], bf16, tag="hn")

                    # logits psum: bias pair opens the accumulation
                    pl = psum_l.tile([128, OF], f32, tag="pl")
                    nc.tensor.matmul(pl[0:B, :], onesrow, boutrow[:, 0:OF],
                                     start=True, stop=False,
                                     skip_group_check=True)
                    nc.tensor.matmul(pl[B:128, :], onesrow,
                                     boutrow[:, OF:O],
                                     start=True, stop=False,
                                     skip_group_check=True)

                    hT_new = state_ht.tile([128, KH * B], bf16, tag="ht")
                    HQ = HF // 2
                    for q in range(2):
                        cs = slice(q * HQ, (q + 1) * HQ)
                        nc.vector.tensor_mul(out=u1[:, cs],
                                             in0=sg[:, 1, cs],
                                             in1=c_prev[:, cs])
                        nc.vector.tensor_add(out=c_new[:, cs],
                                             in0=u1[:, cs], in1=u2[:, cs])
                        nc.scalar.activation(out=th[:, cs],
                                             in_=c_new[:, cs], func=Tanh)
                        nc.vector.tensor_mul(out=hn[:, cs],
                                             in0=sg[:, 2, cs],
                                             in1=th[:, cs])
                        # two [128,128] transposes per half; each yields
                        # hT column-blocks (2ch, 2ch+1) = k-tiles (ch, ch+4)
                        ptrs = {}
                        for ch in (2 * q, 2 * q + 1):
                            ptrh = psum_t.tile([128, 128], bf16, tag="ptr")
                            nc.tensor.transpose(
                                ptrh, hn[:, ch * 128:(ch + 1) * 128], ident)
                            ptrs[ch] = ptrh
                            _emit_logits(nc, ptrs, hT_new, pl, wout, ch)

                    # ---------------- softmax (folded) ----------------
                    eu = work.tile([128, OF], f32, tag="eu")
                    ssum = work.tile([128, 1], f32, tag="ssum")
                    nc.scalar.activation(out=eu, in_=pl, func=Exp,
                                         accum_out=ssum)
                    stmp = work.tile([B, 1], f32, tag="stmp")
                    nc.vector.tensor_copy(out=stmp, in_=ssum[B:128])
                    sden = work.tile([B, 1], f32, tag="sden")
                    nc.vector.tensor_add(out=sden, in0=ssum[0:B],
                                         in1=stmp)
                    sinv = work.tile([128, 1], f32, tag="sinv")
                    nc.vector.reciprocal(out=sinv[0:B], in_=sden)
                    nc.vector.reciprocal(out=sinv[B:128], in_=sden)
                    y = ys.tile([128, OF], f32, tag="y")
                    nc.scalar.mul(out=y[:, 0:128], in_=eu[:, 0:128],
                                  mul=sinv)
                    nc.scalar.mul(out=y[:, 128:OF], in_=eu[:, 128:OF],
                                  mul=sinv)
                    trow = (T - 1 - t) % T
                    nc.sync.dma_start(out=out_d[trow, :, 0:OF],
                                      in_=y[0:B, :])
                    nc.sync.dma_start(out=out_d[trow, :, OF:O],
                                      in_=y[B:128, :])
                    if t < t_live - 1 or TIMING_REPS:
                        ybf = ys.tile([128, OF], bf16, tag="ybf")
                        nc.scalar.mul(out=ybf, in_=eu, mul=sinv)
                        pend_ybf = ybf

                    c_prev = c_new
                    hT_prev = hT_new

    nc.compile()
    return nc


def _host_prep(h0, c0, W_ih, W_hh, b_ih, b_hh, W_out, b_out):
    """Build per-core input maps (host-side layout transforms)."""
    f32 = np.float32
    h0 = np.asarray(h0, f32).reshape(NCORES * B, H)
    c0 = np.asarray(c0, f32).reshape(NCORES * B, H)
    W_ih = np.asarray(W_ih, f32)
    W_hh = np.asarray(W_hh, f32)
    W_out = np.asarray(W_out, f32)
    b_tot = np.asarray(b_ih, f32) + np.asarray(b_hh, f32)
    b_out = np.asarray(b_out, f32)

    # permute gate order (i, f, g, o) -> (i, f, o, g)
    perm = np.r_[0:H, H:2 * H, 3 * H:4 * H, 2 * H:3 * H]
    Wih_p = W_ih[perm]          # [G, O]
    Whh_p = W_hh[perm]          # [G, H]
    b_p = b_tot[perm]           # [G]

    # weight layout: [p, kpos, gate, h-half, h-col] flattened to [128, K*G],
    # with k-tiles permuted so transposed activations land in order.
    WihT_aug = Wih_p.T + b_p[None, :]           # [O, G]
    wih_host = np.ascontiguousarray(
        WihT_aug.reshape(KX, 128, 4, 2, HF)[KXORDER].transpose(1, 0, 2, 3, 4)
    ).reshape(128, KX * G).astype(_BF16)
    whh_host = np.ascontiguousarray(
        Whh_p.T.reshape(KH, 128, 4, 2, HF)[KORDER].transpose(1, 0, 2, 3, 4)
    ).reshape(128, KH * G).astype(_BF16)
    # wout blocks: [kpos, O-half, O-col]
    wout_host = np.ascontiguousarray(
        W_out.T.reshape(KH, 128, O)[KORDER].transpose(1, 0, 2)
    ).reshape(128, KH * O).astype(_BF16)
    biasrow = b_p[None, :].astype(_BF16)        # [1, (gate, half, col)]
    boutrow = b_out[None, :].astype(_BF16)
    onesrow = np.ones((1, B), _BF16)
    ident = np.eye(128).astype(_BF16)

    in_maps = []
    for i in range(NCORES):
        sl = slice(i * B, (i + 1) * B)
        h0s = h0[sl]                                # [B, H]
        h0t = np.ascontiguousarray(
            h0s.reshape(B, KH, 128).transpose(1, 2, 0)[KORDER]
        ).reshape(KH, 128, B).transpose(1, 0, 2).reshape(128, KH * B)
        c0f = np.ascontiguousarray(
            c0[sl].reshape(B, 2, HF).transpose(1, 0, 2)).reshape(128, HF)
        in_maps.append({
            "wih": wih_host, "whh": whh_host, "wout": wout_host,
            "biasrow": biasrow, "boutrow": boutrow, "onesrow": onesrow,
            "ident": ident,
            "h0t": np.ascontiguousarray(h0t).astype(_BF16),
            "c0": c0f,
        })
    return in_maps


def kernel(h0, c0, W_ih, W_hh, b_ih, b_hh, W_out, b_out, out_len):
    from concourse.bass_utils import run_bass_kernel_spmd

    assert int(out_len) == T
    if "nc" not in _cache:
        _cache["nc"] = _build()
    nc = _cache["nc"]
    in_maps = _host_prep(h0, c0, W_ih, W_hh, b_ih, b_hh, W_out, b_out)
    res = run_bass_kernel_spmd(nc, in_maps, core_ids=list(range(NCORES)))
    full = np.empty((T, NCORES * B, O), np.float32)
    for i in range(NCORES):
        full[:, i * B:(i + 1) * B, :] = res.results[i]["out"]
    return full
